# revision 1
# baseline (speedup 1.0000x reference)
"""Causal self-attention on 8 Trainium2 NeuronCores.

Sharding: batch (2) x head-groups (4 heads each) -> 8 cores. Each core
computes Q/K/V projections for its 4 heads, causal attention, and the
partial output projection for its head rows of Wo. The host sums the 4
partials per batch (the "all-reduce" of the row-sharded Wo done on host
during the gather step).

Device-side layout is fully transposed: QT/KT [m, s] come straight out of
W-stationary matmuls, scoresT [sk, sq] feed an augmented-V matmul whose
extra ones-column produces the softmax denominator for free, and the
normalized attendedT [m, s] is exactly the stationary operand the output
projection wants. The causal mask is applied as a multiplicative
upper-triangular 128x128 block on the diagonal score chunks; off-diagonal
masked chunks are never computed.

All matmul operands live in float32r (TF32-like, 1 PE cycle/row vs 4 for
fp32); PSUM accumulation stays fp32.
"""

from contextlib import ExitStack

import numpy as np

import concourse.bacc as bacc
import concourse.bass as bass  # noqa: F401  (AP helpers)
import concourse.mybir as mybir
import concourse.tile as tile
from concourse.bass_utils import run_bass_kernel_spmd

P = 128
B, S, D, H, HD = 2, 2048, 1024, 16, 64
NCORES = 8
HC = 4            # heads per core
MC = HC * HD      # 256 output columns (m) per core
VW = HC * (HD + 1)  # V'' width: 4 heads x (64 vals + 1 ones col)
NDC = D // P      # 8 contraction chunks
NST = S // P      # 16 sequence tiles
F32 = mybir.dt.float32
R32 = mybir.dt.float32r

_NC_CACHE = None


def _pieces(c0, c1, step=512):
    """Split [c0, c1) at `step`-aligned boundaries (PSUM-bank-safe matmuls)."""
    out = []
    c = c0
    while c < c1:
        n = min(c1, (c // step + 1) * step)
        out.append((c, n))
        c = n
    return out


def _build_program():
    nc = bacc.Bacc("TRN2", target_bir_lowering=False, debug=False)
    xt = nc.dram_tensor("xt", [D, S], R32, kind="ExternalInput").ap()
    wq = nc.dram_tensor("wq", [D, MC], R32, kind="ExternalInput").ap()
    wk = nc.dram_tensor("wk", [D, MC], R32, kind="ExternalInput").ap()
    wv = nc.dram_tensor("wv", [D, VW], R32, kind="ExternalInput").ap()
    wo = nc.dram_tensor("wo", [MC, D], R32, kind="ExternalInput").ap()
    tri = nc.dram_tensor("tri", [P, P], R32, kind="ExternalInput").ap()
    out = nc.dram_tensor("out", [S, D], F32, kind="ExternalOutput").ap()

    with tile.TileContext(nc) as tc, ExitStack() as ctx, \
            nc.allow_low_precision(reason="float32r matmul pipeline"):
        constp = ctx.enter_context(tc.tile_pool(name="constp", bufs=1))
        xtp = ctx.enter_context(tc.tile_pool(name="xtp", bufs=1))
        kxp = ctx.enter_context(tc.tile_pool(name="kxp", bufs=1))
        wp = ctx.enter_context(tc.tile_pool(name="wp", bufs=1))
        qkp = ctx.enter_context(tc.tile_pool(name="qkp", bufs=1))
        vp = ctx.enter_context(tc.tile_pool(name="vp", bufs=1))
        attp = ctx.enter_context(tc.tile_pool(name="attp", bufs=1))
        expp = ctx.enter_context(tc.tile_pool(name="expp", bufs=2))
        outp = ctx.enter_context(tc.tile_pool(name="outp", bufs=2))
        drp = ctx.enter_context(tc.tile_pool(name="drp", bufs=1))
        ps = ctx.enter_context(tc.tile_pool(name="ps", bufs=4, space="PSUM"))

        # constants: causal-keep mask (tri[r,c] = r<=c) + a ones row for the
        # denominator broadcast matmul, packed into one tile
        trio = constp.tile([P, P + 64], R32)
        nc.sync.dma_start(trio[:, 0:P], tri)
        # memset can't target f32r; write the 1.0f bit pattern as uint32.
        # ones rows 0..64 so the denominator broadcast matmul can take its
        # stationary at partition base 0 (sub A) or 64 (sub B).
        ONE_BITS = 0x3F800000
        nc.vector.memset(trio[0:65, P:P + 64].bitcast(mybir.dt.uint32),
                         ONE_BITS)
        tri_sb = trio[:, 0:P]

        # interleave x chunks with their weights so dc-k matmuls unblock
        # progressively instead of after the full 10.9MB ingest
        wq_sb = wp.tile([P, NDC, MC], R32)
        wk_sb = wp.tile([P, NDC, MC], R32)
        wv_sb = wp.tile([P, NDC, VW], R32)
        wo_sb = wp.tile([P, 2, D], R32)
        xt_sb = xtp.tile([P, NDC, S], R32)
        for dc in range(NDC):
            nc.sync.dma_start(xt_sb[:, dc, :], xt[dc * P:(dc + 1) * P, :])
            nc.sync.dma_start(wq_sb[:, dc, :], wq[dc * P:(dc + 1) * P, :])
            nc.sync.dma_start(wk_sb[:, dc, :], wk[dc * P:(dc + 1) * P, :])
            nc.sync.dma_start(wv_sb[:, dc, :], wv[dc * P:(dc + 1) * P, :])
        for mc2 in range(2):
            nc.sync.dma_start(wo_sb[:, mc2, :], wo[mc2 * P:(mc2 + 1) * P, :])

        # ---- projections: QT/KT [m, s] (W stationary), V natural [s, m'] ----
        # KT goes straight into a per-head layout padded to full 128
        # contraction rows (zeros in the other head's rows). A 64-row
        # stationary never registers as PE activity in the HAM window, so
        # the clock gate held the whole attention phase at 1.2GHz. The
        # moving qt rows of the other head hit the zero weights, so
        # results are unchanged.
        qt_sb = qkp.tile([P, 2, S], R32)
        kt_pad = kxp.tile([P, HC, S], R32)
        v_sb = vp.tile([P, NST, VW], R32)
        for hh in range(HC):
            zo = 64 - (hh % 2) * 64  # the other head's rows: zeros
            nc.vector.memset(
                kt_pad[zo:zo + 64, hh, :].bitcast(mybir.dt.uint32), 0)
        # Q and K share one psum tile (Q cols 0:512, K cols 512:1024) so all
        # four sequence slabs accumulate concurrently with dc outermost —
        # the PE consumes each x chunk as its DMA lands instead of stalling
        # on the full ingest.
        for mc2 in range(2):
            pqks = [ps.tile([P, 1024], F32, tag="ps", name=f"pqk{s_}")
                    for s_ in range(4)]
            for dc in range(NDC):
                for slab in range(4):
                    s0 = slab * 512
                    nc.tensor.matmul(pqks[slab][:, 0:512],
                                     wq_sb[:, dc, mc2 * P:(mc2 + 1) * P],
                                     xt_sb[:, dc, s0:s0 + 512],
                                     start=(dc == 0), stop=(dc == NDC - 1))
                    nc.tensor.matmul(pqks[slab][:, 512:1024],
                                     wk_sb[:, dc, mc2 * P:(mc2 + 1) * P],
                                     xt_sb[:, dc, s0:s0 + 512],
                                     start=(dc == 0), stop=(dc == NDC - 1))
            for slab in range(4):
                s0 = slab * 512
                pqk = pqks[slab]
                nc.vector.tensor_copy(qt_sb[:, mc2, s0:s0 + 512],
                                      pqk[:, 0:512])
                nc.vector.tensor_copy(kt_pad[0:64, 2 * mc2, s0:s0 + 512],
                                      pqk[0:64, 512:1024])
                nc.vector.tensor_copy(kt_pad[64:128, 2 * mc2 + 1,
                                             s0:s0 + 512],
                                      pqk[64:128, 512:1024])
        def emit_vproj(st_range):
            for st in st_range:
                pv = ps.tile([P, VW], F32, tag="ps")
                for dc in range(NDC):
                    nc.tensor.matmul(pv[:, :],
                                     xt_sb[:, dc, st * P:(st + 1) * P],
                                     wv_sb[:, dc, :],
                                     start=(dc == 0), stop=(dc == NDC - 1))
                nc.vector.tensor_copy(v_sb[:, st, :], pv[:, :])

        emit_vproj(range(8))
        for j in range(HC):
            nc.vector.memset(
                v_sb[:, 0:8, j * (HD + 1) + HD].bitcast(mybir.dt.uint32),
                ONE_BITS)

        def finish_v():
            for j in range(HC):
                nc.vector.memset(
                    v_sb[:, 8:NST, j * (HD + 1) + HD].bitcast(mybir.dt.uint32),
                    ONE_BITS)

        # ---- attention: two heads interleaved to keep the PE dense ----
        # (single-head chains stall the PE on the exp round-trip; the HAM
        # clock gate then never re-warms and the whole phase runs at 1.2GHz)
        att_sb = attp.tile([P, 2, S], R32)
        pending = []  # deferred normalize broadcasts (see below)

        def flush_pending():
            # The pb broadcast matmul waits on the DVE recip chain; emitted
            # at its own segment boundary it stalls the in-order PE stream
            # (and a >3.4us PE gap re-throttles the HAM clock to 1.2GHz).
            # Deferred one segment, dr is long ready and the PE absorbs it
            # between attended matmuls with no stall.
            while pending:
                asl_p, dr_p, row = pending.pop(0)
                pb = ps.tile([64, 1024], F32, tag="ps")
                for (a, b) in _pieces(0, 1024):
                    nc.tensor.matmul(pb[:, a:b],
                                     trio[row:row + 1, P:P + 64],
                                     dr_p[:, a:b],
                                     start=True, stop=True)
                nc.vector.tensor_mul(asl_p, asl_p, pb[:, :])

        def emit_outproj(st_range):
            # out[s, :] = attT.T @ Wo_c for the given sequence tiles
            for st in st_range:
                po = ps.tile([P, 1024], F32, tag="ps")
                for mc2 in (1, 0):
                    for (a, b) in _pieces(0, 1024):
                        nc.tensor.matmul(po[:, a:b],
                                         att_sb[:, mc2, st * P:(st + 1) * P],
                                         wo_sb[:, mc2, a:b],
                                         start=(mc2 == 1), stop=(mc2 == 0))
                ot = outp.tile([P, 1024], F32)
                nc.vector.tensor_copy(ot[:, :], po[:, :])
                nc.sync.dma_start(out[st * P:(st + 1) * P, :], ot[:, :])

        # half-major: after both mcq segments of half 0, sq tiles 0..7 are
        # fully attended, so their output projection is injected into the
        # half-1 chunk stream as guaranteed-ready PE filler work
        for half in range(2):
            hbase = half * 1024
            nch = (half + 1) * 8  # causal: sk chunks 0 .. sq_max/128
            # last chunk touching each 512-col psum bank (for stop flags)
            last_t = {0: max(i for i in range(nch)
                             if max(0, i * P - hbase) < 512),
                      1: nch - 1}
            for mcq in ([0, 1] if half == 0 else [1, 0]):
                pas = [ps.tile([P, 1024], F32, tag="ps", name=f"pa{s_}")
                       for s_ in range(2)]
                for i in range(nch):
                    if i == 2:
                        # by now the previous segment's recip chain is done
                        # and the PE absorbs its broadcast without stalling
                        flush_pending()
                    if half == 0 and 3 <= i <= 6:
                        # V projection for the second-half sk tiles doubles
                        # as full-array PE filler during these chunks
                        st0 = 8 + 4 * mcq + (i - 3)
                        emit_vproj([st0])
                        if mcq == 1 and i == 6:
                            finish_v()
                    if half == 1 and 3 <= i <= 6:
                        # sq tiles 0..7 are fully attended after half 0:
                        # their output projection is ready PE filler for
                        # both half-1 segments (mcq 1 runs first)
                        emit_outproj([i - 3 if mcq == 1 else i + 1])
                    c0 = max(0, i * P - hbase)  # first valid sq col (local)
                    for sub in range(2):
                        hh = 2 * mcq + sub
                        poff = sub * 64
                        vlo = hh * (HD + 1)
                        pa = pas[sub]
                        pscr = ps.tile([P, 1024], F32, tag="ps")
                        for (a, b) in _pieces(c0, 1024):
                            nc.tensor.matmul(
                                pscr[:, a:b],
                                kt_pad[:, hh, i * P:(i + 1) * P],
                                qt_sb[:, mcq, hbase + a:hbase + b],
                                start=True, stop=True)
                        et = expp.tile([P, 1024], R32)
                        nc.scalar.activation(
                            out=et[:, c0:1024], in_=pscr[:, c0:1024],
                            func=mybir.ActivationFunctionType.Exp,
                            scale=0.125)
                        if i * P >= hbase:  # diagonal block: zero sk > sq
                            nc.vector.tensor_mul(et[:, c0:c0 + P],
                                                 et[:, c0:c0 + P], tri_sb)
                        for (a, b) in _pieces(c0, 1024):
                            nc.tensor.matmul(
                                pa[0:HD + 1, a:b],
                                v_sb[:, i, vlo:vlo + HD + 1],
                                et[:, a:b],
                                start=(i == 0), stop=(i == last_t[a // 512]))
                # normalize: row HD of pa is the softmax denominator.
                # Drain both pa tiles first so their PSUM slots free for
                # the next segment before the recip chain runs. All ops on
                # DVE at partition base 0 (ACT lanes are partition-locked
                # and the custom recip op misbehaves off base 0 on HW).
                dr65 = drp.tile([65, 1024], R32)
                dd = drp.tile([1, 2048], F32)
                for sub in range(2):
                    poff = sub * 64
                    pa = pas[sub]
                    asl = att_sb[poff:poff + 64, mcq, hbase:hbase + 1024]
                    if sub == 0:
                        # partition-aligned (0:64 -> 0:64): ACT may copy it,
                        # overlapping the DVE boundary chain
                        nc.scalar.copy(asl, pa[0:64, :])
                    else:
                        nc.vector.tensor_copy(asl, pa[0:64, :])
                    nc.vector.tensor_copy(dd[:, sub * 1024:sub * 1024 + 1024],
                                          pa[HD:HD + 1, :])
                    pending.append((asl, dr65[poff:poff + 1, :], poff))
                for sub in range(2):
                    sl = dd[:, sub * 1024:sub * 1024 + 1024]
                    nc.vector.reciprocal_approx_fast(out=sl, in_=sl)
                    nc.vector.tensor_copy(dr65[sub * 64:sub * 64 + 1, :], sl)
        flush_pending()
        emit_outproj(range(8, NST))

    nc.compile()
    return nc


def get_program():
    global _NC_CACHE
    if _NC_CACHE is None:
        _NC_CACHE = _build_program()
    return _NC_CACHE


def prepare_in_maps(inputs):
    x = np.asarray(inputs["x"], dtype=np.float32)
    Wq = np.asarray(inputs["Wq"], dtype=np.float32)
    Wk = np.asarray(inputs["Wk"], dtype=np.float32)
    Wv = np.asarray(inputs["Wv"], dtype=np.float32)
    Wo = np.asarray(inputs["Wo"], dtype=np.float32)
    xts = [np.ascontiguousarray(x[b].T) for b in range(B)]
    tri = np.triu(np.ones((P, P), dtype=np.float32))
    in_maps = []
    for c in range(NCORES):
        b = c // 4
        hg = c % 4
        cols = slice(hg * MC, (hg + 1) * MC)
        wv_c = np.zeros((D, VW), np.float32)
        for j in range(HC):
            wv_c[:, j * (HD + 1):j * (HD + 1) + HD] = \
                Wv[:, hg * MC + j * HD:hg * MC + (j + 1) * HD]
        in_maps.append({
            "xt": xts[b],
            "wq": np.ascontiguousarray(Wq[:, cols]),
            "wk": np.ascontiguousarray(Wk[:, cols]),
            "wv": wv_c,
            "wo": np.ascontiguousarray(Wo[cols, :]),
            "tri": tri,
        })
    return in_maps


def gather_output(results):
    outs = [np.asarray(results[c]["out"], dtype=np.float32)
            for c in range(NCORES)]
    return np.stack([outs[0] + outs[1] + outs[2] + outs[3],
                     outs[4] + outs[5] + outs[6] + outs[7]])


def kernel(**inputs) -> np.ndarray:
    nc = get_program()
    in_maps = prepare_in_maps(inputs)
    res = run_bass_kernel_spmd(nc, in_maps, list(range(NCORES)))
    return gather_output(res.results)



# revision 10
# speedup vs baseline: 1.1366x; 1.1366x over previous
"""Causal self-attention on 8 Trainium2 NeuronCores (v2, bf16 pipeline).

Sharding: batch (2) x head-groups (4 heads each) -> 8 cores. Each core
computes Q/K/V projections for its 4 heads, causal attention, and the
partial output projection for its head rows of Wo. The host sums the 4
partials per batch (the "all-reduce" of the row-sharded Wo done on host
during the gather step).

v2 changes vs the f32r baseline (259us):
- bf16 operands end-to-end (PSUM stays f32): halves DMA and SBUF
  traffic; PE stream rate is the same 1 col/cycle as f32r.
- single host-packed ingest tensor [D, 2820] = xt|wq|wk|wv, one DMA per
  128-row contraction chunk (11 descriptors total vs 35; each ~610ns of
  serial issue on the sync queue).
- kt psum drains moved to ACT (partition-aligned, idle during the
  projection phase); qt/v drains stay on DVE.
- all memsets (kt_pad zero-padding, V ones columns, pb ones rows) on
  GpSimd, off the DVE/ACT critical path.
- boundary chain slimmed: denominator rows gathered into one [2,1024]
  tile, ONE reciprocal op per segment (was two), recip result cast to
  f32r for the pb broadcast matmul.
- output in bf16 (host up-converts and sums in f32): halves the tail
  DMA; out-tile psum drains alternate ACT/DVE in the tail.
- output DMA issued from the GpSimd queue (sync queue is the ingest
  bottleneck at ~610ns/descriptor).

Device-side layout is fully transposed: QT/KT [m, s] come straight out
of W-stationary matmuls, scoresT [sk, sq] feed an augmented-V matmul
whose extra ones-column produces the softmax denominator for free, and
the normalized attendedT [m, s] is exactly the stationary operand the
output projection wants. The causal mask is applied as a multiplicative
upper-triangular 128x128 block on the diagonal score chunks;
off-diagonal masked chunks are never computed.
"""

from contextlib import ExitStack

import numpy as np
import ml_dtypes

import concourse.bacc as bacc
import concourse.bass as bass  # noqa: F401  (AP helpers)
import concourse.mybir as mybir
import concourse.tile as tile
from concourse.bass_utils import run_bass_kernel_spmd

P = 128
B, S, D, H, HD = 2, 2048, 1024, 16, 64
NCORES = 8
HC = 4            # heads per core
MC = HC * HD      # 256 output columns (m) per core
VW = HC * (HD + 1)  # V'' width: 4 heads x (64 vals + 1 ones col)
NDC = D // P      # 8 contraction chunks
NST = S // P      # 16 sequence tiles
F32 = mybir.dt.float32
R32 = mybir.dt.float32r
BF = mybir.dt.bfloat16
NPBF = ml_dtypes.bfloat16

# ingest packing offsets (columns of the [D, IW] host tensor)
QO = S            # 2048
KO = QO + MC      # 2304
VO = KO + MC      # 2560
IW = VO + VW      # 2820

_NC_CACHE = None


def _pieces(c0, c1, step=512):
    """Split [c0, c1) at `step`-aligned boundaries (PSUM-bank-safe matmuls)."""
    out = []
    c = c0
    while c < c1:
        n = min(c1, (c // step + 1) * step)
        out.append((c, n))
        c = n
    return out


def _build_program():
    nc = bacc.Bacc("TRN2", target_bir_lowering=False, debug=False)
    ing = nc.dram_tensor("ing", [D, IW], BF, kind="ExternalInput").ap()
    wo = nc.dram_tensor("wo", [MC, D], BF, kind="ExternalInput").ap()
    tri = nc.dram_tensor("tri", [P, P], BF, kind="ExternalInput").ap()
    out = nc.dram_tensor("out", [S, D], BF, kind="ExternalOutput").ap()

    with tile.TileContext(nc) as tc, ExitStack() as ctx, \
            nc.allow_low_precision(reason="bf16 matmul pipeline"):
        constp = ctx.enter_context(tc.tile_pool(name="constp", bufs=1))
        xtp = ctx.enter_context(tc.tile_pool(name="xtp", bufs=1))
        kxp = ctx.enter_context(tc.tile_pool(name="kxp", bufs=1))
        wp = ctx.enter_context(tc.tile_pool(name="wp", bufs=1))
        qkp = ctx.enter_context(tc.tile_pool(name="qkp", bufs=1))
        vp = ctx.enter_context(tc.tile_pool(name="vp", bufs=1))
        attp = ctx.enter_context(tc.tile_pool(name="attp", bufs=1))
        expp = ctx.enter_context(tc.tile_pool(name="expp", bufs=2))
        outp = ctx.enter_context(tc.tile_pool(name="outp", bufs=2))
        drp = ctx.enter_context(tc.tile_pool(name="drp", bufs=1))
        ps = ctx.enter_context(tc.tile_pool(name="ps", bufs=4, space="PSUM"))

        # ---- constants + ingest -------------------------------------
        ONE_BITS = 0x3F800000
        trio = constp.tile([P, P], BF)
        nc.sync.dma_start(trio[:, :], tri)
        tri_sb = trio[:, 0:P]
        # ones rows for the denominator broadcast matmul (f32r so the pb
        # matmul runs at 1 col/cycle with the f32r recip rows). Matmul
        # operands must sit at base partition 0/32/64, so the two per-sub
        # denominator rows live at partitions 0 and 32.
        ones_r = constp.tile([33, HD], R32)
        nc.gpsimd.memset(ones_r[:, :].bitcast(mybir.dt.uint32), ONE_BITS)

        ing_sb = xtp.tile([P, NDC, IW], BF)
        for dc in range(NDC):
            nc.sync.dma_start(ing_sb[:, dc, :], ing[dc * P:(dc + 1) * P, :])
        wo_sb = wp.tile([P, 2, D], BF)
        for mc2 in range(2):
            nc.sync.dma_start(wo_sb[:, mc2, :], wo[mc2 * P:(mc2 + 1) * P, :])

        def xt_of(dc):
            return ing_sb[:, dc, 0:S]

        # KT goes straight into a per-head layout padded to full 128
        # contraction rows (zeros in the other head's rows). A 64-row
        # stationary never registers as PE activity in the HAM window, so
        # the clock gate held the whole attention phase at 1.2GHz. The
        # moving qt rows of the other head hit the zero weights, so
        # results are unchanged. Zero-padding runs on GpSimd (idle).
        kt_pad = kxp.tile([P, HC, S], BF)
        for hh in range(HC):
            zo = 64 - (hh % 2) * 64  # the other head's rows: zeros
            nc.gpsimd.memset(kt_pad[zo:zo + 64, hh, :], 0.0)
        v_sb = vp.tile([P, NST, VW], BF)

        # ---- projections: QT/KT [m, s] (W stationary), V natural ----
        # Q and K share one psum tile (Q cols 0:512, K cols 512:1024) so all
        # four sequence slabs accumulate concurrently with dc outermost —
        # the PE consumes each x chunk as its DMA lands instead of stalling
        # on the full ingest.
        qt_sb = qkp.tile([P, 2, S], BF)
        for mc2 in range(2):
            pqks = [ps.tile([P, 1024], F32, tag="ps", name=f"pqk{s_}")
                    for s_ in range(4)]
            for dc in range(NDC):
                for slab in range(4):
                    s0 = slab * 512
                    nc.tensor.matmul(pqks[slab][:, 0:512],
                                     ing_sb[:, dc, QO + mc2 * P:
                                            QO + (mc2 + 1) * P],
                                     xt_of(dc)[:, s0:s0 + 512],
                                     start=(dc == 0), stop=(dc == NDC - 1))
                    nc.tensor.matmul(pqks[slab][:, 512:1024],
                                     ing_sb[:, dc, KO + mc2 * P:
                                            KO + (mc2 + 1) * P],
                                     xt_of(dc)[:, s0:s0 + 512],
                                     start=(dc == 0), stop=(dc == NDC - 1))
            for slab in range(4):
                s0 = slab * 512
                pqk = pqks[slab]
                # qt drain on DVE; kt drains on ACT (partition-aligned,
                # and ACT is otherwise idle until the attention phase)
                nc.vector.tensor_copy(qt_sb[:, mc2, s0:s0 + 512],
                                      pqk[:, 0:512])
                nc.scalar.copy(kt_pad[0:64, 2 * mc2, s0:s0 + 512],
                               pqk[0:64, 512:1024])
                nc.scalar.copy(kt_pad[64:128, 2 * mc2 + 1, s0:s0 + 512],
                               pqk[64:128, 512:1024])

        def emit_vproj(st_range):
            for st in st_range:
                pv = ps.tile([P, VW], F32, tag="ps")
                for dc in range(NDC):
                    nc.tensor.matmul(pv[:, :],
                                     ing_sb[:, dc, st * P:(st + 1) * P],
                                     ing_sb[:, dc, VO:VO + VW],
                                     start=(dc == 0), stop=(dc == NDC - 1))
                nc.vector.tensor_copy(v_sb[:, st, :], pv[:, :])

        emit_vproj(range(8))
        # the vproj drain writes the full 260-col slab (zeros land in the
        # ones columns from the host's zero-padded wv), so the ones
        # memsets must FOLLOW the drains. GpSimd keeps them off DVE/ACT.
        for j in range(HC):
            nc.gpsimd.memset(v_sb[:, 0:8, j * (HD + 1) + HD], 1.0)

        def finish_v():
            for j in range(HC):
                nc.gpsimd.memset(v_sb[:, 8:NST, j * (HD + 1) + HD], 1.0)

        # ---- attention: two heads interleaved to keep the PE dense ----
        # (single-head chains stall the PE on the exp round-trip; the HAM
        # clock gate then never re-warms and the whole phase runs at 1.2GHz)
        att_sb = attp.tile([P, 2, S], BF)
        dd = drp.tile([33, 1024], F32)
        dr2 = drp.tile([33, 1024], R32)
        # rows 1..31 of dd are never written; pre-fill so the recip over
        # [0:33] (cost is column-driven, rows are free) sees finite junk
        nc.gpsimd.memset(dd[:, :], 1.0)
        pending = []  # deferred normalize broadcasts (see below)

        def flush_pending():
            # The pb broadcast matmul waits on the DVE recip chain; emitted
            # at its own segment boundary it stalls the in-order PE stream
            # (and a >3.4us PE gap re-throttles the HAM clock to 1.2GHz).
            # Deferred one segment, dr2 is long ready and the PE absorbs it
            # between attended matmuls with no stall.
            while pending:
                asl0, asl1 = pending.pop(0)
                pb = ps.tile([64, 1024], F32, tag="ps")
                for row, asl in ((0, asl0), (32, asl1)):
                    for (a, b) in _pieces(0, 1024):
                        nc.tensor.matmul(pb[:, a:b],
                                         ones_r[row:row + 1, :],
                                         dr2[row:row + 1, a:b],
                                         start=True, stop=True)
                    nc.vector.tensor_mul(asl, asl, pb[:, :])

        def emit_outproj(st_range, tail=False):
            # out[s, :] = attT.T @ Wo_c for the given sequence tiles
            for st in st_range:
                po = ps.tile([P, 1024], F32, tag="ps")
                for mc2 in (1, 0):
                    for (a, b) in _pieces(0, 1024):
                        nc.tensor.matmul(po[:, a:b],
                                         att_sb[:, mc2, st * P:(st + 1) * P],
                                         wo_sb[:, mc2, a:b],
                                         start=(mc2 == 1), stop=(mc2 == 0))
                ot = outp.tile([P, 1024], BF)
                if tail and st % 2 == 0:
                    nc.scalar.copy(ot[:, :], po[:, :])
                else:
                    nc.vector.tensor_copy(ot[:, :], po[:, :])
                nc.gpsimd.dma_start(out[st * P:(st + 1) * P, :], ot[:, :])

        # half-major: after both mcq segments of half 0, sq tiles 0..7 are
        # fully attended, so their output projection is injected into the
        # half-1 chunk stream as guaranteed-ready PE filler work
        for half in range(2):
            hbase = half * 1024
            nch = (half + 1) * 8  # causal: sk chunks 0 .. sq_max/128
            # last chunk touching each 512-col psum bank (for stop flags)
            last_t = {0: max(i for i in range(nch)
                             if max(0, i * P - hbase) < 512),
                      1: nch - 1}
            for mcq in ([0, 1] if half == 0 else [1, 0]):
                # the flush's pb matmul waits on the recip chain (~5us
                # after the previous segment's last matmul); defer it far
                # enough into this segment's chunk stream that the PE
                # never stalls on it. In half1-mcq1 the outproj fillers
                # read half0's att, which flush normalizes — flush must
                # precede them (fillers shifted to i 4..7 there).
                flush_i = 3 if (half == 1 and mcq == 1) else 4
                pas = [ps.tile([P, 1024], F32, tag="ps", name=f"pa{s_}")
                       for s_ in range(2)]
                for i in range(nch):
                    if i == flush_i:
                        flush_pending()
                    if half == 0 and 3 <= i <= 6:
                        # V projection for the second-half sk tiles doubles
                        # as full-array PE filler during these chunks
                        st0 = 8 + 4 * mcq + (i - 3)
                        emit_vproj([st0])
                        if mcq == 1 and i == 6:
                            finish_v()
                    if half == 1 and mcq == 1 and 4 <= i <= 7:
                        # sq tiles 0..7 are fully attended after half 0:
                        # their output projection is ready PE filler for
                        # both half-1 segments (mcq 1 runs first)
                        emit_outproj([i - 4])
                    if half == 1 and mcq == 0 and 3 <= i <= 6:
                        emit_outproj([i + 1])
                    c0 = max(0, i * P - hbase)  # first valid sq col (local)
                    for sub in range(2):
                        hh = 2 * mcq + sub
                        vlo = hh * (HD + 1)
                        pa = pas[sub]
                        pscr = ps.tile([P, 1024], F32, tag="ps")
                        for (a, b) in _pieces(c0, 1024):
                            nc.tensor.matmul(
                                pscr[:, a:b],
                                kt_pad[:, hh, i * P:(i + 1) * P],
                                qt_sb[:, mcq, hbase + a:hbase + b],
                                start=True, stop=True)
                        et = expp.tile([P, 1024], BF)
                        nc.scalar.activation(
                            out=et[:, c0:1024], in_=pscr[:, c0:1024],
                            func=mybir.ActivationFunctionType.Exp,
                            scale=0.125)
                        if i * P >= hbase:  # diagonal block: zero sk > sq
                            nc.vector.tensor_mul(et[:, c0:c0 + P],
                                                 et[:, c0:c0 + P], tri_sb)
                        for (a, b) in _pieces(c0, 1024):
                            nc.tensor.matmul(
                                pa[0:HD + 1, a:b],
                                v_sb[:, i, vlo:vlo + HD + 1],
                                et[:, a:b],
                                start=(i == 0), stop=(i == last_t[a // 512]))
                # normalize: row HD of pa is the softmax denominator.
                # Drain both pa tiles first so their PSUM slots free for
                # the next segment before the recip chain runs. The recip
                # runs once on the packed [2, 1024] denominator tile (DVE
                # at partition base 0 — ACT lanes are partition-locked and
                # the custom recip op misbehaves off base 0 on HW).
                for sub in range(2):
                    nc.vector.tensor_copy(dd[32 * sub:32 * sub + 1, :],
                                          pas[sub][HD:HD + 1, :])
                nc.vector.reciprocal_approx_fast(out=dd, in_=dd)
                nc.vector.tensor_copy(dr2[:, :], dd[:, :])
                asls = []
                for sub in range(2):
                    poff = sub * 64
                    asl = att_sb[poff:poff + 64, mcq, hbase:hbase + 1024]
                    if sub == 0:
                        # partition-aligned (0:64 -> 0:64): ACT may copy it,
                        # overlapping the DVE recip chain
                        nc.scalar.copy(asl, pas[sub][0:64, :])
                    else:
                        nc.vector.tensor_copy(asl, pas[sub][0:64, :])
                    asls.append(asl)
                pending.append(tuple(asls))
        flush_pending()
        emit_outproj(range(8, NST), tail=True)

    nc.compile()
    return nc


def get_program():
    global _NC_CACHE
    if _NC_CACHE is None:
        _NC_CACHE = _build_program()
    return _NC_CACHE


def prepare_in_maps(inputs):
    x = np.asarray(inputs["x"], dtype=np.float32)
    Wq = np.asarray(inputs["Wq"], dtype=np.float32)
    Wk = np.asarray(inputs["Wk"], dtype=np.float32)
    Wv = np.asarray(inputs["Wv"], dtype=np.float32)
    Wo = np.asarray(inputs["Wo"], dtype=np.float32)
    xts = [np.ascontiguousarray(x[b].T) for b in range(B)]
    tri = np.triu(np.ones((P, P), dtype=np.float32)).astype(NPBF)
    in_maps = []
    for c in range(NCORES):
        b = c // 4
        hg = c % 4
        cols = slice(hg * MC, (hg + 1) * MC)
        wv_c = np.zeros((D, VW), np.float32)
        for j in range(HC):
            wv_c[:, j * (HD + 1):j * (HD + 1) + HD] = \
                Wv[:, hg * MC + j * HD:hg * MC + (j + 1) * HD]
        ing = np.concatenate(
            [xts[b], Wq[:, cols], Wk[:, cols], wv_c], axis=1).astype(NPBF)
        in_maps.append({
            "ing": np.ascontiguousarray(ing),
            "wo": np.ascontiguousarray(Wo[cols, :]).astype(NPBF),
            "tri": tri,
        })
    return in_maps


def gather_output(results):
    outs = [np.asarray(results[c]["out"]).astype(np.float32)
            for c in range(NCORES)]
    return np.stack([outs[0] + outs[1] + outs[2] + outs[3],
                     outs[4] + outs[5] + outs[6] + outs[7]])


def kernel(**inputs) -> np.ndarray:
    nc = get_program()
    in_maps = prepare_in_maps(inputs)
    res = run_bass_kernel_spmd(nc, in_maps, list(range(NCORES)))
    return gather_output(res.results)


# revision 13
# speedup vs baseline: 1.3445x; 1.1829x over previous
"""Causal self-attention on 8 Trainium2 NeuronCores (v2, bf16 pipeline).

Sharding: batch (2) x head-groups (4 heads each) -> 8 cores. Each core
computes Q/K/V projections for its 4 heads, causal attention, and the
partial output projection for its head rows of Wo. The host sums the 4
partials per batch (the "all-reduce" of the row-sharded Wo done on host
during the gather step).

v2 changes vs the f32r baseline (259us):
- bf16 operands end-to-end (PSUM stays f32): halves DMA and SBUF
  traffic; PE stream rate is the same 1 col/cycle as f32r.
- single host-packed ingest tensor [D, 2820] = xt|wq|wk|wv, one DMA per
  128-row contraction chunk (11 descriptors total vs 35; each ~610ns of
  serial issue on the sync queue).
- kt psum drains moved to ACT (partition-aligned, idle during the
  projection phase); qt/v drains stay on DVE.
- all memsets (kt_pad zero-padding, V ones columns, pb ones rows) on
  GpSimd, off the DVE/ACT critical path.
- boundary chain slimmed: denominator rows gathered into one [2,1024]
  tile, ONE reciprocal op per segment (was two), recip result cast to
  f32r for the pb broadcast matmul.
- output in bf16 (host up-converts and sums in f32): halves the tail
  DMA; out-tile psum drains alternate ACT/DVE in the tail.
- output DMA issued from the GpSimd queue (sync queue is the ingest
  bottleneck at ~610ns/descriptor).

Device-side layout is fully transposed: QT/KT [m, s] come straight out
of W-stationary matmuls, scoresT [sk, sq] feed an augmented-V matmul
whose extra ones-column produces the softmax denominator for free, and
the normalized attendedT [m, s] is exactly the stationary operand the
output projection wants. The causal mask is applied as a multiplicative
upper-triangular 128x128 block on the diagonal score chunks;
off-diagonal masked chunks are never computed.
"""

from contextlib import ExitStack

import numpy as np
import ml_dtypes

import concourse.bacc as bacc
import concourse.bass as bass  # noqa: F401  (AP helpers)
import concourse.mybir as mybir
import concourse.tile as tile
from concourse.bass_utils import run_bass_kernel_spmd

P = 128
B, S, D, H, HD = 2, 2048, 1024, 16, 64
NCORES = 8
HC = 4            # heads per core
MC = HC * HD      # 256 output columns (m) per core
VW = HC * (HD + 1)  # V'' width: 4 heads x (64 vals + 1 ones col)
NDC = D // P      # 8 contraction chunks
NST = S // P      # 16 sequence tiles
F32 = mybir.dt.float32
R32 = mybir.dt.float32r
BF = mybir.dt.bfloat16
NPBF = ml_dtypes.bfloat16

# ingest packing offsets (columns of the [D, IW] host tensor)
QO = S            # 2048
KO = QO + MC      # 2304
VO = KO + MC      # 2560
IW = VO + VW      # 2820

_NC_CACHE = None


def _pieces(c0, c1, step=512):
    """Split [c0, c1) at `step`-aligned boundaries (PSUM-bank-safe matmuls)."""
    out = []
    c = c0
    while c < c1:
        n = min(c1, (c // step + 1) * step)
        out.append((c, n))
        c = n
    return out


def _build_program():
    nc = bacc.Bacc("TRN2", target_bir_lowering=False, debug=False)
    ing = nc.dram_tensor("ing", [D, IW], BF, kind="ExternalInput").ap()
    wo = nc.dram_tensor("wo", [MC, D], BF, kind="ExternalInput").ap()
    tri = nc.dram_tensor("tri", [P, P], BF, kind="ExternalInput").ap()
    out = nc.dram_tensor("out", [S, D], BF, kind="ExternalOutput").ap()

    with tile.TileContext(nc) as tc, ExitStack() as ctx, \
            nc.allow_low_precision(reason="bf16 matmul pipeline"):
        constp = ctx.enter_context(tc.tile_pool(name="constp", bufs=1))
        xtp = ctx.enter_context(tc.tile_pool(name="xtp", bufs=1))
        kxp = ctx.enter_context(tc.tile_pool(name="kxp", bufs=1))
        wp = ctx.enter_context(tc.tile_pool(name="wp", bufs=1))
        qkp = ctx.enter_context(tc.tile_pool(name="qkp", bufs=1))
        vp = ctx.enter_context(tc.tile_pool(name="vp", bufs=1))
        attp = ctx.enter_context(tc.tile_pool(name="attp", bufs=1))
        expp = ctx.enter_context(tc.tile_pool(name="expp", bufs=4))
        outp = ctx.enter_context(tc.tile_pool(name="outp", bufs=2))
        drp = ctx.enter_context(tc.tile_pool(name="drp", bufs=1))
        ps = ctx.enter_context(tc.tile_pool(name="ps", bufs=4, space="PSUM"))

        # ---- constants + ingest -------------------------------------
        ONE_BITS = 0x3F800000
        trio = constp.tile([P, P], BF)
        nc.sync.dma_start(trio[:, :], tri)
        tri_sb = trio[:, 0:P]
        # ones rows for the denominator broadcast matmul (f32r so the pb
        # matmul runs at 1 col/cycle with the f32r recip rows). Matmul
        # operands must sit at base partition 0/32/64, so the two per-sub
        # denominator rows live at partitions 0 and 32.
        ones_r = constp.tile([33, HD], R32)
        nc.gpsimd.memset(ones_r[:, :].bitcast(mybir.dt.uint32), ONE_BITS)

        ing_sb = xtp.tile([P, NDC, IW], BF)
        for dc in range(NDC):
            nc.sync.dma_start(ing_sb[:, dc, :], ing[dc * P:(dc + 1) * P, :])
        wo_sb = wp.tile([P, 2, D], BF)
        for mc2 in range(2):
            nc.sync.dma_start(wo_sb[:, mc2, :], wo[mc2 * P:(mc2 + 1) * P, :])

        def xt_of(dc):
            return ing_sb[:, dc, 0:S]

        # KT goes straight into a per-head layout padded to full 128
        # contraction rows (zeros in the other head's rows). A 64-row
        # stationary never registers as PE activity in the HAM window, so
        # the clock gate held the whole attention phase at 1.2GHz. The
        # moving qt rows of the other head hit the zero weights, so
        # results are unchanged. Zero-padding runs on GpSimd (idle).
        kt_pad = kxp.tile([P, HC, S], BF)
        for hh in range(HC):
            zo = 64 - (hh % 2) * 64  # the other head's rows: zeros
            nc.gpsimd.memset(kt_pad[zo:zo + 64, hh, :], 0.0)
        v_sb = vp.tile([P, NST, VW], BF)

        # ---- projections: QT/KT [m, s] (W stationary), V natural ----
        # Q and K share one psum tile (Q cols 0:512, K cols 512:1024) so all
        # four sequence slabs accumulate concurrently with dc outermost —
        # the PE consumes each x chunk as its DMA lands instead of stalling
        # on the full ingest.
        qt_sb = qkp.tile([P, 2, S], BF)
        for mc2 in range(2):
            pqks = [ps.tile([P, 1024], F32, tag="ps", name=f"pqk{s_}")
                    for s_ in range(4)]
            for dc in range(NDC):
                for slab in range(4):
                    s0 = slab * 512
                    nc.tensor.matmul(pqks[slab][:, 0:512],
                                     ing_sb[:, dc, QO + mc2 * P:
                                            QO + (mc2 + 1) * P],
                                     xt_of(dc)[:, s0:s0 + 512],
                                     start=(dc == 0), stop=(dc == NDC - 1))
                    nc.tensor.matmul(pqks[slab][:, 512:1024],
                                     ing_sb[:, dc, KO + mc2 * P:
                                            KO + (mc2 + 1) * P],
                                     xt_of(dc)[:, s0:s0 + 512],
                                     start=(dc == 0), stop=(dc == NDC - 1))
            for slab in range(4):
                s0 = slab * 512
                pqk = pqks[slab]
                # qt drain on DVE; kt drains split ACT/DVE so each slab's
                # psum slot frees in one drain-latency (the mc2 transition
                # stalls the PE on the slowest slab drain)
                nc.vector.tensor_copy(qt_sb[:, mc2, s0:s0 + 512],
                                      pqk[:, 0:512])
                nc.scalar.copy(kt_pad[0:64, 2 * mc2, s0:s0 + 512],
                               pqk[0:64, 512:1024])
                nc.vector.tensor_copy(kt_pad[64:128, 2 * mc2 + 1,
                                             s0:s0 + 512],
                                      pqk[64:128, 512:1024])

        def emit_vproj(st_range):
            for st in st_range:
                pv = ps.tile([P, VW], F32, tag="ps")
                for dc in range(NDC):
                    nc.tensor.matmul(pv[:, :],
                                     ing_sb[:, dc, st * P:(st + 1) * P],
                                     ing_sb[:, dc, VO:VO + VW],
                                     start=(dc == 0), stop=(dc == NDC - 1))
                nc.vector.tensor_copy(v_sb[:, st, :], pv[:, :])

        emit_vproj(range(8))
        # the vproj drain writes the full 260-col slab (zeros land in the
        # ones columns from the host's zero-padded wv), so the ones
        # memsets must FOLLOW the drains. GpSimd keeps them off DVE/ACT.
        for j in range(HC):
            nc.gpsimd.memset(v_sb[:, 0:8, j * (HD + 1) + HD], 1.0)

        def finish_v():
            for j in range(HC):
                nc.gpsimd.memset(v_sb[:, 8:NST, j * (HD + 1) + HD], 1.0)

        # ---- attention: two heads interleaved to keep the PE dense ----
        # (single-head chains stall the PE on the exp round-trip; the HAM
        # clock gate then never re-warms and the whole phase runs at 1.2GHz)
        att_sb = attp.tile([P, 2, S], BF)
        dd = drp.tile([33, 1024], F32)
        dr2 = drp.tile([33, 1024], R32)
        # rows 1..31 of dd are never written; pre-fill so the recip over
        # [0:33] (cost is column-driven, rows are free) sees finite junk
        nc.gpsimd.memset(dd[:, :], 1.0)
        pending = []  # deferred normalize broadcasts (see below)

        def flush_pending():
            # The pb broadcast matmul waits on the DVE recip chain; emitted
            # at its own segment boundary it stalls the in-order PE stream
            # (and a >3.4us PE gap re-throttles the HAM clock to 1.2GHz).
            # Deferred one segment, dr2 is long ready and the PE absorbs it
            # between attended matmuls with no stall.
            while pending:
                asl0, asl1 = pending.pop(0)
                pb = ps.tile([64, 1024], F32, tag="ps")
                for row, asl in ((0, asl0), (32, asl1)):
                    for (a, b) in _pieces(0, 1024):
                        nc.tensor.matmul(pb[:, a:b],
                                         ones_r[row:row + 1, :],
                                         dr2[row:row + 1, a:b],
                                         start=True, stop=True)
                    nc.vector.tensor_mul(asl, asl, pb[:, :])

        def emit_outproj(st_range, tail=False):
            # out[s, :] = attT.T @ Wo_c for the given sequence tiles
            for st in st_range:
                po = ps.tile([P, 1024], F32, tag="ps")
                for mc2 in (1, 0):
                    for (a, b) in _pieces(0, 1024):
                        nc.tensor.matmul(po[:, a:b],
                                         att_sb[:, mc2, st * P:(st + 1) * P],
                                         wo_sb[:, mc2, a:b],
                                         start=(mc2 == 1), stop=(mc2 == 0))
                ot = outp.tile([P, 1024], BF)
                if tail and st % 2 == 0:
                    nc.scalar.copy(ot[:, :], po[:, :])
                else:
                    nc.vector.tensor_copy(ot[:, :], po[:, :])
                nc.gpsimd.dma_start(out[st * P:(st + 1) * P, :], ot[:, :])

        # half-major: after both mcq segments of half 0, sq tiles 0..7 are
        # fully attended, so their output projection is injected into the
        # half-1 chunk stream as guaranteed-ready PE filler work
        for half in range(2):
            hbase = half * 1024
            nch = (half + 1) * 8  # causal: sk chunks 0 .. sq_max/128
            # last chunk touching each 512-col psum bank (for stop flags)
            last_t = {0: max(i for i in range(nch)
                             if max(0, i * P - hbase) < 512),
                      1: nch - 1}
            for mcq in ([0, 1] if half == 0 else [1, 0]):
                pas = [ps.tile([P, 1024], F32, tag="ps", name=f"pa{s_}")
                       for s_ in range(2)]

                def emit_scores(i):
                    # scores + exp for chunk i, both heads; returns the et
                    # tiles the deferred attended matmuls will consume
                    c0 = max(0, i * P - hbase)
                    ets = []
                    for sub in range(2):
                        hh = 2 * mcq + sub
                        pscr = ps.tile([P, 1024], F32, tag="ps")
                        for (a, b) in _pieces(c0, 1024):
                            nc.tensor.matmul(
                                pscr[:, a:b],
                                kt_pad[:, hh, i * P:(i + 1) * P],
                                qt_sb[:, mcq, hbase + a:hbase + b],
                                start=True, stop=True)
                        et = expp.tile([P, 1024], BF)
                        nc.scalar.activation(
                            out=et[:, c0:1024], in_=pscr[:, c0:1024],
                            func=mybir.ActivationFunctionType.Exp,
                            scale=0.125)
                        if i * P >= hbase:  # diagonal block: zero sk > sq
                            nc.vector.tensor_mul(et[:, c0:c0 + P],
                                                 et[:, c0:c0 + P], tri_sb)
                        ets.append(et)
                    return ets

                def emit_attended(i, ets):
                    c0 = max(0, i * P - hbase)
                    for sub in range(2):
                        vlo = (2 * mcq + sub) * (HD + 1)
                        for (a, b) in _pieces(c0, 1024):
                            nc.tensor.matmul(
                                pas[sub][0:HD + 1, a:b],
                                v_sb[:, i, vlo:vlo + HD + 1],
                                ets[sub][:, a:b],
                                start=(i == 0), stop=(i == last_t[a // 512]))

                # software-pipelined chunk loop: scores for chunk i+1 are
                # emitted BEFORE attended of chunk i, so the in-order PE
                # queue always has the next scores ready when the attended
                # matmuls block on the exp round-trip — ACT (the attention
                # bottleneck at 1.2GHz) never starves behind a blocked PE.
                ets_prev = emit_scores(0)
                for i in range(1, nch + 1):
                    if i == 4:
                        # previous segment's recip chain is ~done; the PE
                        # absorbs its broadcast without stalling
                        flush_pending()
                    if half == 0 and 3 <= i <= 6:
                        # V projection for the second-half sk tiles doubles
                        # as full-array PE filler during these chunks
                        st0 = 8 + 4 * mcq + (i - 3)
                        emit_vproj([st0])
                        if mcq == 1 and i == 6:
                            finish_v()
                    if half == 1 and mcq == 1 and 5 <= i <= 8:
                        # sq tiles 0..7 are fully attended after half 0:
                        # their output projection is ready PE filler for
                        # both half-1 segments (mcq 1 runs first)
                        emit_outproj([i - 5])
                    if half == 1 and mcq == 0 and 4 <= i <= 7:
                        emit_outproj([i])
                    if i < nch:
                        ets_next = emit_scores(i)
                    emit_attended(i - 1, ets_prev)
                    if i < nch:
                        ets_prev = ets_next
                # normalize: row HD of pa is the softmax denominator.
                # Drain both pa tiles first (DVE — ACT is the attention
                # bottleneck) so their PSUM slots free for the next
                # segment, then run the recip chain. The recip runs once
                # on the packed [33, 1024] denominator tile (DVE at
                # partition base 0 — the custom recip op misbehaves off
                # base 0 on HW; rows 0/32 so the pb matmul operands sit at
                # legal base partitions).
                asls = []
                for sub in range(2):
                    poff = sub * 64
                    asl = att_sb[poff:poff + 64, mcq, hbase:hbase + 1024]
                    nc.vector.tensor_copy(asl, pas[sub][0:64, :])
                    asls.append(asl)
                for sub in range(2):
                    nc.vector.tensor_copy(dd[32 * sub:32 * sub + 1, :],
                                          pas[sub][HD:HD + 1, :])
                nc.vector.reciprocal_approx_fast(out=dd, in_=dd)
                nc.vector.tensor_copy(dr2[:, :], dd[:, :])
                pending.append(tuple(asls))
        flush_pending()
        emit_outproj(range(8, NST), tail=True)

    nc.compile()
    return nc


def get_program():
    global _NC_CACHE
    if _NC_CACHE is None:
        _NC_CACHE = _build_program()
    return _NC_CACHE


def prepare_in_maps(inputs):
    x = np.asarray(inputs["x"], dtype=np.float32)
    Wq = np.asarray(inputs["Wq"], dtype=np.float32)
    Wk = np.asarray(inputs["Wk"], dtype=np.float32)
    Wv = np.asarray(inputs["Wv"], dtype=np.float32)
    Wo = np.asarray(inputs["Wo"], dtype=np.float32)
    xts = [np.ascontiguousarray(x[b].T) for b in range(B)]
    tri = np.triu(np.ones((P, P), dtype=np.float32)).astype(NPBF)
    in_maps = []
    for c in range(NCORES):
        b = c // 4
        hg = c % 4
        cols = slice(hg * MC, (hg + 1) * MC)
        wv_c = np.zeros((D, VW), np.float32)
        for j in range(HC):
            wv_c[:, j * (HD + 1):j * (HD + 1) + HD] = \
                Wv[:, hg * MC + j * HD:hg * MC + (j + 1) * HD]
        ing = np.concatenate(
            [xts[b], Wq[:, cols], Wk[:, cols], wv_c], axis=1).astype(NPBF)
        in_maps.append({
            "ing": np.ascontiguousarray(ing),
            "wo": np.ascontiguousarray(Wo[cols, :]).astype(NPBF),
            "tri": tri,
        })
    return in_maps


def gather_output(results):
    outs = [np.asarray(results[c]["out"]).astype(np.float32)
            for c in range(NCORES)]
    return np.stack([outs[0] + outs[1] + outs[2] + outs[3],
                     outs[4] + outs[5] + outs[6] + outs[7]])


def kernel(**inputs) -> np.ndarray:
    nc = get_program()
    in_maps = prepare_in_maps(inputs)
    res = run_bass_kernel_spmd(nc, in_maps, list(range(NCORES)))
    return gather_output(res.results)


# revision 20
# speedup vs baseline: 1.3769x; 1.0241x over previous
"""Causal self-attention on 8 Trainium2 NeuronCores (v2, bf16 pipeline).

Sharding: batch (2) x head-groups (4 heads each) -> 8 cores. Each core
computes Q/K/V projections for its 4 heads, causal attention, and the
partial output projection for its head rows of Wo. The host sums the 4
partials per batch (the "all-reduce" of the row-sharded Wo done on host
during the gather step).

v2 changes vs the f32r baseline (259us):
- bf16 operands end-to-end (PSUM stays f32): halves DMA and SBUF
  traffic; PE stream rate is the same 1 col/cycle as f32r.
- single host-packed ingest tensor [D, 2820] = xt|wq|wk|wv, one DMA per
  128-row contraction chunk (11 descriptors total vs 35; each ~610ns of
  serial issue on the sync queue).
- kt psum drains moved to ACT (partition-aligned, idle during the
  projection phase); qt/v drains stay on DVE.
- all memsets (kt_pad zero-padding, V ones columns, pb ones rows) on
  GpSimd, off the DVE/ACT critical path.
- boundary chain slimmed: denominator rows gathered into one [2,1024]
  tile, ONE reciprocal op per segment (was two), recip result cast to
  f32r for the pb broadcast matmul.
- output in bf16 (host up-converts and sums in f32): halves the tail
  DMA; out-tile psum drains alternate ACT/DVE in the tail.
- output DMA issued from the GpSimd queue (sync queue is the ingest
  bottleneck at ~610ns/descriptor).

Device-side layout is fully transposed: QT/KT [m, s] come straight out
of W-stationary matmuls, scoresT [sk, sq] feed an augmented-V matmul
whose extra ones-column produces the softmax denominator for free, and
the normalized attendedT [m, s] is exactly the stationary operand the
output projection wants. The causal mask is applied as a multiplicative
upper-triangular 128x128 block on the diagonal score chunks;
off-diagonal masked chunks are never computed.
"""

from contextlib import ExitStack

import numpy as np
import ml_dtypes

import concourse.bacc as bacc
import concourse.bass as bass  # noqa: F401  (AP helpers)
import concourse.mybir as mybir
import concourse.tile as tile
from concourse.bass_utils import run_bass_kernel_spmd

P = 128
B, S, D, H, HD = 2, 2048, 1024, 16, 64
NCORES = 8
HC = 4            # heads per core
MC = HC * HD      # 256 output columns (m) per core
VW = HC * (HD + 1)  # V'' width: 4 heads x (64 vals + 1 ones col)
NDC = D // P      # 8 contraction chunks
NST = S // P      # 16 sequence tiles
F32 = mybir.dt.float32
R32 = mybir.dt.float32r
BF = mybir.dt.bfloat16
NPBF = ml_dtypes.bfloat16

# ingest packing offsets (columns of the [D, IW] host tensor)
QO = S            # 2048
KO = QO + MC      # 2304
VO = KO + MC      # 2560
IW = VO + VW      # 2820

_NC_CACHE = None


def _pieces(c0, c1, step=512):
    """Split [c0, c1) at `step`-aligned boundaries (PSUM-bank-safe matmuls)."""
    out = []
    c = c0
    while c < c1:
        n = min(c1, (c // step + 1) * step)
        out.append((c, n))
        c = n
    return out


def _build_program():
    nc = bacc.Bacc("TRN2", target_bir_lowering=False, debug=False)
    ing = nc.dram_tensor("ing", [D, IW], BF, kind="ExternalInput").ap()
    wo = nc.dram_tensor("wo", [MC, D], BF, kind="ExternalInput").ap()
    tri = nc.dram_tensor("tri", [P, P], BF, kind="ExternalInput").ap()
    out = nc.dram_tensor("out", [S, D], BF, kind="ExternalOutput").ap()

    with tile.TileContext(nc) as tc, ExitStack() as ctx, \
            nc.allow_low_precision(reason="bf16 matmul pipeline"):
        constp = ctx.enter_context(tc.tile_pool(name="constp", bufs=1))
        xtp = ctx.enter_context(tc.tile_pool(name="xtp", bufs=1))
        kxp = ctx.enter_context(tc.tile_pool(name="kxp", bufs=1))
        wp = ctx.enter_context(tc.tile_pool(name="wp", bufs=1))
        qkp = ctx.enter_context(tc.tile_pool(name="qkp", bufs=1))
        vp = ctx.enter_context(tc.tile_pool(name="vp", bufs=1))
        attp = ctx.enter_context(tc.tile_pool(name="attp", bufs=1))
        expp = ctx.enter_context(tc.tile_pool(name="expp", bufs=4))
        outp = ctx.enter_context(tc.tile_pool(name="outp", bufs=2))
        drp = ctx.enter_context(tc.tile_pool(name="drp", bufs=1))
        ps = ctx.enter_context(tc.tile_pool(name="ps", bufs=4, space="PSUM"))

        # ---- constants + ingest -------------------------------------
        # ing chunk 0 is issued FIRST: the sync queue posts descriptors
        # serially (~610ns each) and the first QK matmul waits on it
        ing_sb = xtp.tile([P, NDC, IW], BF)
        for dc in range(NDC):
            nc.sync.dma_start(ing_sb[:, dc, :], ing[dc * P:(dc + 1) * P, :])
        trio = constp.tile([P, P], BF)
        nc.sync.dma_start(trio[:, :], tri)
        tri_sb = trio[:, 0:P]
        # ones rows for the denominator broadcast matmul (bf16 so the pb
        # matmul streams at 1 col/cycle — f32r moving ran in fp32 mode at
        # 4 cycles/col). Matmul operands must sit at base partition
        # 0/32/64, so the two per-sub denominator rows live at 0 and 32.
        ones_b = constp.tile([33, HD], BF)
        nc.gpsimd.memset(ones_b[:, :], 1.0)
        wo_sb = wp.tile([P, 2, D], BF)
        for mc2 in range(2):
            nc.sync.dma_start(wo_sb[:, mc2, :], wo[mc2 * P:(mc2 + 1) * P, :])

        def xt_of(dc):
            return ing_sb[:, dc, 0:S]

        # KT goes straight into a per-head layout padded to full 128
        # contraction rows (zeros in the other head's rows). A 64-row
        # stationary never registers as PE activity in the HAM window, so
        # the clock gate held the whole attention phase at 1.2GHz. The
        # moving qt rows of the other head hit the zero weights, so
        # results are unchanged. Zero-padding runs on GpSimd (idle).
        kt_pad = kxp.tile([P, HC, S], BF)
        for hh in range(HC):
            zo = 64 - (hh % 2) * 64  # the other head's rows: zeros
            nc.gpsimd.memset(kt_pad[zo:zo + 64, hh, :], 0.0)
        v_sb = vp.tile([P, NST, VW], BF)

        # ---- projections: QT/KT [m, s] (W stationary), V natural ----
        # Q and K share one psum tile (Q cols 0:512, K cols 512:1024) so all
        # four sequence slabs accumulate concurrently with dc outermost —
        # the PE consumes each x chunk as its DMA lands instead of stalling
        # on the full ingest.
        qt_sb = qkp.tile([P, 2, S], BF)
        for mc2 in range(2):
            pqks = [ps.tile([P, 1024], F32, tag="ps", name=f"pqk{s_}")
                    for s_ in range(4)]
            for dc in range(NDC):
                for slab in range(4):
                    s0 = slab * 512
                    nc.tensor.matmul(pqks[slab][:, 0:512],
                                     ing_sb[:, dc, QO + mc2 * P:
                                            QO + (mc2 + 1) * P],
                                     xt_of(dc)[:, s0:s0 + 512],
                                     start=(dc == 0), stop=(dc == NDC - 1))
                    nc.tensor.matmul(pqks[slab][:, 512:1024],
                                     ing_sb[:, dc, KO + mc2 * P:
                                            KO + (mc2 + 1) * P],
                                     xt_of(dc)[:, s0:s0 + 512],
                                     start=(dc == 0), stop=(dc == NDC - 1))
            for slab in range(4):
                s0 = slab * 512
                pqk = pqks[slab]
                # qt drain on DVE; kt drains split ACT/DVE so each slab's
                # psum slot frees in one drain-latency (the mc2 transition
                # stalls the PE on the slowest slab drain)
                nc.vector.tensor_copy(qt_sb[:, mc2, s0:s0 + 512],
                                      pqk[:, 0:512])
                nc.scalar.copy(kt_pad[0:64, 2 * mc2, s0:s0 + 512],
                               pqk[0:64, 512:1024])
                nc.vector.tensor_copy(kt_pad[64:128, 2 * mc2 + 1,
                                             s0:s0 + 512],
                                      pqk[64:128, 512:1024])

        def emit_vproj(st_range):
            for st in st_range:
                pv = ps.tile([P, VW], F32, tag="ps")
                for dc in range(NDC):
                    nc.tensor.matmul(pv[:, :],
                                     ing_sb[:, dc, st * P:(st + 1) * P],
                                     ing_sb[:, dc, VO:VO + VW],
                                     start=(dc == 0), stop=(dc == NDC - 1))
                nc.vector.tensor_copy(v_sb[:, st, :], pv[:, :])
                # the drain writes the full 260-col slab (zeros land in
                # the ones columns from the host's zero-padded wv), so the
                # ones memset must FOLLOW it. GpSimd keeps it off DVE/ACT;
                # the stepped slice hits all 4 heads' ones columns at once.
                nc.gpsimd.memset(v_sb[:, st, HD:VW:HD + 1], 1.0)

        # only st 0/1 before the attention stream starts; the rest are
        # emitted inside the chunk loop as PE filler (attention is
        # ACT-bound, so the PE has idle slots to absorb them)
        emit_vproj([0, 1])

        # ---- attention: two heads interleaved to keep the PE dense ----
        # (single-head chains stall the PE on the exp round-trip; the HAM
        # clock gate then never re-warms and the whole phase runs at 1.2GHz)
        att_sb = attp.tile([P, 2, S], BF)
        dd = drp.tile([33, 1024], F32)
        dr2 = drp.tile([33, 1024], BF)
        # rows 1..31 of dd are never written; pre-fill so the recip over
        # [0:33] (cost is column-driven, rows are free) sees finite junk
        nc.gpsimd.memset(dd[:, :], 1.0)
        pending = []  # deferred normalize broadcasts (see below)

        def flush_pending():
            # The pb broadcast matmul waits on the DVE recip chain; emitted
            # at its own segment boundary it stalls the in-order PE stream
            # (and a >3.4us PE gap re-throttles the HAM clock to 1.2GHz).
            # Deferred, dr2 is long ready and the PE absorbs it between
            # attended matmuls with no stall.
            while pending:
                mcq_, hb_, a, b = pending.pop(0)
                w = b - a
                pb = ps.tile([P, 512], F32, tag="ps", name="pb")
                for sub in range(2):
                    nc.tensor.matmul(pb[64 * sub:64 * sub + 64, 0:w],
                                     ones_b[32 * sub:32 * sub + 1, :],
                                     dr2[32 * sub:32 * sub + 1, a:b],
                                     start=True, stop=True)
                for sub in range(2):
                    asl = att_sb[64 * sub:64 * sub + 64, mcq_,
                                 hb_ + a:hb_ + b]
                    nc.vector.tensor_mul(asl, asl,
                                         pb[64 * sub:64 * sub + 64, 0:w])

        def finalize_piece(pas, mcq, hbase, a, b):
            # att drain + denominator gather + recip for one 512-col psum
            # bank of the pa accumulators. Bank 0 completes at last_t[0] —
            # chunks before segment end — so its chain hides behind the
            # remaining chunk stream instead of stalling the boundary.
            for sub in range(2):
                asl = att_sb[64 * sub:64 * sub + 64, mcq,
                             hbase + a:hbase + b]
                nc.vector.tensor_copy(asl, pas[sub][0:64, a:b])
            for sub in range(2):
                nc.vector.tensor_copy(dd[32 * sub:32 * sub + 1, a:b],
                                      pas[sub][HD:HD + 1, a:b])
            nc.vector.reciprocal_approx_fast(out=dd[:, a:b], in_=dd[:, a:b])
            nc.vector.tensor_copy(dr2[:, a:b], dd[:, a:b])
            pending.append((mcq, hbase, a, b))

        def emit_outproj(st_range, tail=False):
            # out[s, :] = attT.T @ Wo_c for the given sequence tiles
            for st in st_range:
                po = ps.tile([P, 1024], F32, tag="ps")
                for mc2 in (1, 0):
                    for (a, b) in _pieces(0, 1024):
                        nc.tensor.matmul(po[:, a:b],
                                         att_sb[:, mc2, st * P:(st + 1) * P],
                                         wo_sb[:, mc2, a:b],
                                         start=(mc2 == 1), stop=(mc2 == 0))
                ot = outp.tile([P, 1024], BF)
                if tail and st % 2 == 0:
                    nc.scalar.copy(ot[:, :], po[:, :])
                else:
                    nc.vector.tensor_copy(ot[:, :], po[:, :])
                nc.gpsimd.dma_start(out[st * P:(st + 1) * P, :], ot[:, :])

        # half-major: after both mcq segments of half 0, sq tiles 0..7 are
        # fully attended, so their output projection is injected into the
        # half-1 chunk stream as guaranteed-ready PE filler work
        for half in range(2):
            hbase = half * 1024
            nch = (half + 1) * 8  # causal: sk chunks 0 .. sq_max/128
            # last chunk touching each 512-col psum bank (for stop flags)
            last_t = {0: max(i for i in range(nch)
                             if max(0, i * P - hbase) < 512),
                      1: nch - 1}
            for mcq in ([0, 1] if half == 0 else [1, 0]):
                is_last = (half == 1 and mcq == 0)
                pas = [ps.tile([P, 1024], F32, tag="ps", name=f"pa{s_}")
                       for s_ in range(2)]

                def emit_scores(i):
                    # scores + exp for chunk i, both heads; returns the et
                    # tiles the deferred attended matmuls will consume
                    c0 = max(0, i * P - hbase)
                    ets = []
                    for sub in range(2):
                        hh = 2 * mcq + sub
                        pscr = ps.tile([P, 1024], F32, tag="ps")
                        for (a, b) in _pieces(c0, 1024):
                            nc.tensor.matmul(
                                pscr[:, a:b],
                                kt_pad[:, hh, i * P:(i + 1) * P],
                                qt_sb[:, mcq, hbase + a:hbase + b],
                                start=True, stop=True)
                        et = expp.tile([P, 1024], BF)
                        nc.scalar.activation(
                            out=et[:, c0:1024], in_=pscr[:, c0:1024],
                            func=mybir.ActivationFunctionType.Exp,
                            scale=0.125)
                        if i * P >= hbase:  # diagonal block: zero sk > sq
                            nc.vector.tensor_mul(et[:, c0:c0 + P],
                                                 et[:, c0:c0 + P], tri_sb)
                        ets.append(et)
                    return ets

                def emit_attended(i, ets):
                    c0 = max(0, i * P - hbase)
                    for sub in range(2):
                        vlo = (2 * mcq + sub) * (HD + 1)
                        for (a, b) in _pieces(c0, 1024):
                            nc.tensor.matmul(
                                pas[sub][0:HD + 1, a:b],
                                v_sb[:, i, vlo:vlo + HD + 1],
                                ets[sub][:, a:b],
                                start=(i == 0), stop=(i == last_t[a // 512]))

                # software-pipelined chunk loop: scores for chunk i+1 are
                # emitted BEFORE the fillers and before attended of chunk
                # i, so the in-order PE queue always feeds ACT (the
                # attention bottleneck at 1.2GHz) first — a filler waiting
                # on a psum slot can no longer starve the exp stream.
                ets_prev = emit_scores(0)
                for i in range(1, nch + 1):
                    if i < nch:
                        ets_next = emit_scores(i)
                    if i == 4:
                        # previous segment's recip chain is ~done; the PE
                        # absorbs its broadcast without stalling
                        flush_pending()
                    if half == 0:
                        # V projections double as PE filler inside the
                        # ACT-bound chunk stream
                        if mcq == 0 and 1 <= i <= 6:
                            emit_vproj([i + 1])
                        if mcq == 0 and 3 <= i <= 6:
                            emit_vproj([8 + (i - 3)])
                        if mcq == 1 and 3 <= i <= 6:
                            emit_vproj([12 + (i - 3)])
                    if half == 1 and mcq == 1 and 5 <= i <= 8:
                        # sq tiles 0..7 are fully attended after half 0:
                        # their output projection is ready PE filler for
                        # both half-1 segments (mcq 1 runs first)
                        emit_outproj([i - 5])
                    if half == 1 and mcq == 0 and 4 <= i <= 7:
                        emit_outproj([i])
                    if is_last and 13 <= i <= 16:
                        # piece-a of this (final) segment was normalized
                        # in-loop below, so sq tiles 8..11 project inside
                        # the remaining chunk stream — only tiles 12..15
                        # are left for the tail
                        emit_outproj([i - 5])
                    emit_attended(i - 1, ets_prev)
                    if i - 1 == last_t[0]:
                        finalize_piece(pas, mcq, hbase, 0, 512)
                        if is_last:
                            flush_pending()
                    if i < nch:
                        ets_prev = ets_next
                finalize_piece(pas, mcq, hbase, 512, 1024)
        flush_pending()
        emit_outproj(range(12, NST), tail=True)

    nc.compile()
    return nc


def get_program():
    global _NC_CACHE
    if _NC_CACHE is None:
        _NC_CACHE = _build_program()
    return _NC_CACHE


def prepare_in_maps(inputs):
    x = np.asarray(inputs["x"], dtype=np.float32)
    Wq = np.asarray(inputs["Wq"], dtype=np.float32)
    Wk = np.asarray(inputs["Wk"], dtype=np.float32)
    Wv = np.asarray(inputs["Wv"], dtype=np.float32)
    Wo = np.asarray(inputs["Wo"], dtype=np.float32)
    xts = [np.ascontiguousarray(x[b].T) for b in range(B)]
    tri = np.triu(np.ones((P, P), dtype=np.float32)).astype(NPBF)
    in_maps = []
    for c in range(NCORES):
        b = c // 4
        hg = c % 4
        cols = slice(hg * MC, (hg + 1) * MC)
        wv_c = np.zeros((D, VW), np.float32)
        for j in range(HC):
            wv_c[:, j * (HD + 1):j * (HD + 1) + HD] = \
                Wv[:, hg * MC + j * HD:hg * MC + (j + 1) * HD]
        ing = np.concatenate(
            [xts[b], Wq[:, cols], Wk[:, cols], wv_c], axis=1).astype(NPBF)
        in_maps.append({
            "ing": np.ascontiguousarray(ing),
            "wo": np.ascontiguousarray(Wo[cols, :]).astype(NPBF),
            "tri": tri,
        })
    return in_maps


def gather_output(results):
    outs = [np.asarray(results[c]["out"]).astype(np.float32)
            for c in range(NCORES)]
    return np.stack([outs[0] + outs[1] + outs[2] + outs[3],
                     outs[4] + outs[5] + outs[6] + outs[7]])


def kernel(**inputs) -> np.ndarray:
    nc = get_program()
    in_maps = prepare_in_maps(inputs)
    res = run_bass_kernel_spmd(nc, in_maps, list(range(NCORES)))
    return gather_output(res.results)
